# revision 15
# baseline (speedup 1.0000x reference)
"""Trainium2 Bass kernel: 16-head MHA (S=2048, D=1024, Dk=Dv=64) on 8 NeuronCores.

Tensor-parallel over heads (2 heads/core). Streaming schedule:

- The S=2048 query range is processed in 4 stripes of 512. Per stripe the
  attention loop walks 16 key t-blocks: scores for BOTH heads are a
  row-tiled pair of K=64 matmuls (head0 on PE rows 0-63, head1 on 64-127)
  that execute concurrently; one Exp activation [128, 1024] covers both
  heads; AV accumulates into per-head [65, 512] PSUM tiles (ones-column
  appended to V so the softmax denominator falls out of the same matmul).
- K/V/Q projections run in fp8e4m3 (weights pre-scaled x64 on the host to
  escape fp8 subnormals; the factor is folded into the exp scale (2^-15)
  and into Wo (x1/64) so no extra instructions are needed) and are
  *streamed into* the attention loop: only K-chunk 0 and Q-stripe 0 run
  before the first exp; everything else is emitted into attention slots.
- AV for slot i is emitted after the score matmuls of slot i+1 so the PE
  never queue-blocks on an exp in flight.
- Softmax normalization: denominator rows of both heads DMA-reshaped to
  [128, 8], one fast DVE reciprocal, DMA broadcast back (all off the
  PE/ACT critical path; the av PSUM banks are released by a single copy).
- Epilogue: dummy warm-up matmuls keep the PE clock at 2.4 GHz through the
  last normalize, then the final output projection rotates over all freed
  PSUM banks.
- Row-sharded Wo: each core computes partial out^T[c, s] for its 128 rows
  of Wo; the host sums the 8 bf16 partials in fp32.
"""

import numpy as np

import concourse.tile as tile_mod
from concourse import bacc, mybir
from concourse.bass_utils import run_bass_kernel_spmd
from concourse.vector_clock import ScopedClock, VectorClock

F32 = mybir.dt.float32
BF16 = mybir.dt.bfloat16
FP8 = mybir.dt.float8e3
ADD = mybir.AluOpType.add
MULT = mybir.AluOpType.mult
EXP = mybir.ActivationFunctionType.Exp

S, D, H, DK = 2048, 1024, 16, 64
P = 128
NCORES = 8
SW = 512          # stripe width (query positions per attention pass)
NSTRIPE = S // SW
WSCALE = 64.0     # host pre-scale on Wq/Wk/Wv to keep fp8 weights normal
EXP_SCALE = 0.125 / (WSCALE * WSCALE)   # = 2^-15, exact in fp32


def _patched_drain_and_barrier(self, tick_clock, wait_clock):
    """This container's walrus build caps CTRL-type instructions at one sem
    wait, but Tile's exit drain carries one wait per outstanding proc. Emit
    one Drain per outstanding proc instead, each with a single wait."""
    gc = tick_clock.global_clock
    vec = list(gc)
    for i, t in enumerate(vec):
        if t <= 0:
            continue
        pv = [0] * len(vec)
        pv[i] = t
        d = self.nc.sync.drain()
        wait_clock.add_sem_waits(d.ins, ScopedClock({None: VectorClock(pv)}))

    self.nc.all_engine_barrier()
    assert self.sems is not None
    popped = self.nc._tile_sem_poison_stack.pop()
    assert popped is self._sem_poison
    self.nc.clear_and_free_semaphores(list(self.sems.allocated().values()))
    self.nc.all_engine_barrier()


tile_mod.TileContext._drain_and_barrier = _patched_drain_and_barrier


def _build_nc():
    from contextlib import ExitStack

    tile = tile_mod
    nc = bacc.Bacc(None)

    # et_f8: [128, 4 chunks, 8 d-blocks, 512 cols] so each 512-col chunk is
    # one contiguous 4KB-per-partition DMA.  wqkv_f8 column order: K|V|Q.
    et = nc.declare_dram_parameter("et", [P, NSTRIPE * 8 * SW], FP8, isOutput=False)
    wqkv = nc.declare_dram_parameter("wqkv", [P, 8 * 6 * DK], FP8, isOutput=False)
    bqk = nc.declare_dram_parameter("bqk", [P, 2], F32, isOutput=False)
    bv = nc.declare_dram_parameter("bv", [P, 2 * DK], F32, isOutput=False)
    bo = nc.declare_dram_parameter("bo", [P, 8], F32, isOutput=False)
    wo = nc.declare_dram_parameter("wo", [P, D], BF16, isOutput=False)
    out = nc.declare_dram_parameter("out", [D, S], BF16, isOutput=True)

    et4 = et.rearrange("p (c po s) -> p c po s", c=NSTRIPE, po=8)
    wqkv3 = wqkv.rearrange("p (po c) -> p po c", po=8)

    with tile.TileContext(nc) as tc, ExitStack() as ctx:
        consts = ctx.enter_context(tc.tile_pool(name="consts", bufs=1))
        qkv = ctx.enter_context(tc.tile_pool(name="qkv", bufs=1))
        utp = ctx.enter_context(tc.tile_pool(name="ut", bufs=1))
        headsp = ctx.enter_context(tc.tile_pool(name="heads", bufs=1))
        normp = ctx.enter_context(tc.tile_pool(name="norm", bufs=1))
        outp = ctx.enter_context(tc.tile_pool(name="outp", bufs=1))
        psum = ctx.enter_context(tc.tile_pool(name="psum", bufs=1, space="PSUM"))
        dramsm = ctx.enter_context(tc.tile_pool(name="dramsm", bufs=2, space="DRAM"))

        # ---- t=0: trigger the exp table load so it overlaps the input DMAs.
        wu = normp.tile([P, 32], F32, tag="wu")
        nc.vector.memset(wu[:, 0:16], 0.0)
        nc.scalar.activation(wu[:, 16:32], wu[:, 0:16], EXP, scale=EXP_SCALE)

        # ---- input DMAs.  sync: wqkv + et chunks 0,2.  gpsimd: biases,
        # et chunks 1,3, wo (needed latest).
        wqkv_sb = consts.tile([P, 8, 6 * DK], FP8)
        nc.sync.dma_start(wqkv_sb[:, :, 0:128], wqkv3[:, :, 0:128])
        nc.gpsimd.dma_start(wqkv_sb[:, :, 128:384], wqkv3[:, :, 128:384])
        bqk_sb = consts.tile([P, 2], F32)
        nc.gpsimd.dma_start(bqk_sb[:], bqk[:])
        bv_b = consts.tile([P, 2 * DK], F32)
        nc.gpsimd.dma_start(bv_b[:], bv[:])
        bo_c = consts.tile([P, 8], F32)
        nc.gpsimd.dma_start(bo_c[:], bo[:])
        et_sb = consts.tile([P, NSTRIPE, 8, SW], FP8)
        nc.sync.dma_start(et_sb[:, 0], et4[:, 0])
        nc.sync.dma_start(wqkv_sb[:, 1:3], wqkv3[:, 1:3])
        nc.gpsimd.dma_start(et_sb[:, 1], et4[:, 1])
        nc.sync.dma_start(et_sb[:, 2], et4[:, 2])
        nc.gpsimd.dma_start(et_sb[:, 3], et4[:, 3])
        wo_sb = consts.tile([P, D], BF16)
        nc.gpsimd.dma_start(wo_sb[:], wo[:])

        # PE pre-warm: dummy bf16 matmuls (garbage values, discarded) keep
        # the HAM activity window busy so the projections run at 2.4 GHz.
        warm_sb = qkv.tile([P, 640], BF16)
        nc.vector.memset(warm_sb[:], 0.0)

        def emit_warm(n, tag="kq", moving=None, nw=SW):
            for w in range(n):
                wps = psum.tile([P, nw], F32, tag=tag, bufs=1)
                mv = warm_sb[:, 128 : 128 + nw] if moving is None else moving
                nc.tensor.matmul(
                    wps[:], warm_sb[:, 0:128], mv, start=True, stop=True,
                )

        emit_warm(11)

        qt_sb = qkv.tile([P, S], BF16)
        kt_sb = qkv.tile([P, S], BF16)
        vaug_sb = qkv.tile([P, 16, 130], BF16)
        nc.vector.memset(vaug_sb[:, :, 64:65], 1.0)
        nc.vector.memset(vaug_sb[:, :, 129:130], 1.0)

        # PSUM budget (8 banks): st [128,1024] x2 = 4, av [65,512] x2 = 2,
        # kq [128,512] x1 = 1, vop [128,512] x1 = 1.

        def emit_k(c):
            ps = psum.tile([P, SW], F32, tag="kq", bufs=1, name=f"k{c}")
            for dc in range(8):
                nc.tensor.matmul(
                    ps[:],
                    wqkv_sb[:, dc, 0:128],
                    et_sb[:, c, dc, :],
                    start=(dc == 0),
                    stop=(dc == 7),
                )
            nc.vector.tensor_scalar_add(
                kt_sb[:, c * SW : (c + 1) * SW], ps[:], bqk_sb[:, 1:2]
            )

        def emit_q(sg):
            ps = psum.tile([P, SW], F32, tag="vop", bufs=1, name=f"q{sg}")
            for dc in range(8):
                nc.tensor.matmul(
                    ps[:],
                    wqkv_sb[:, dc, 256:384],
                    et_sb[:, sg, dc, :],
                    start=(dc == 0),
                    stop=(dc == 7),
                )
            nc.vector.tensor_scalar_add(
                qt_sb[:, sg * SW : (sg + 1) * SW], ps[:], bqk_sb[:, 0:1]
            )

        def emit_v(j):
            ps = psum.tile([P, SW], F32, tag="vop", bufs=1, name=f"v{j}")
            jj = (j % 4) * P
            for dc in range(8):
                nc.tensor.matmul(
                    ps[:, 0:P],
                    et_sb[:, j // 4, dc, jj : jj + P],
                    wqkv_sb[:, dc, 128:256],
                    start=(dc == 0),
                    stop=(dc == 7),
                )
            nc.vector.tensor_tensor(
                vaug_sb[:, j, 0:64], ps[:, 0:64], bv_b[:, 0:64], ADD
            )
            nc.vector.tensor_tensor(
                vaug_sb[:, j, 65:129], ps[:, 64:128], bv_b[:, 64:128], ADD
            )

        heads_sb = {}

        def emit_op(sg, blk, tag="vop", bufs=1, scalar_bias=False,
                    engs=(nc.sync, nc.gpsimd)):
            c0 = blk * P
            ps = psum.tile([P, SW], F32, tag=tag, bufs=bufs, name=f"op{sg}_{blk}")
            nc.tensor.matmul(
                ps[:], wo_sb[:, c0 : c0 + P], heads_sb[sg][:], start=True, stop=True
            )
            ot = outp.tile([P, SW], BF16, tag="ot", bufs=4)
            if scalar_bias:
                nc.scalar.activation(
                    ot[:], ps[:], mybir.ActivationFunctionType.Identity,
                    bias=bo_c[:, blk : blk + 1],
                )
            else:
                nc.vector.tensor_scalar_add(ot[:], ps[:], bo_c[:, blk : blk + 1])
            eng = engs[blk % len(engs)]
            eng.dma_start(out[c0 : c0 + P, sg * SW : (sg + 1) * SW], ot[:])

        # ---- prologue: only what attention slot 0 needs.
        emit_k(0)
        emit_q(0)

        # Per-stripe production schedules: slot index -> list of units.
        # Deadlines: V_j before the av of slot j (emitted at slot j+1);
        # K_c before the scores of slot 4c; Q_{s+1} before stripe s+1.
        sched = {0: {}, 1: {}, 2: {}, 3: {}}
        for j in range(16):
            sched[0].setdefault(j, []).append(lambda j=j: emit_v(j))
        sched[0][2].insert(0, lambda: emit_k(1))
        sched[0][6].insert(0, lambda: emit_k(2))
        sched[0][10].insert(0, lambda: emit_k(3))
        for pt in range(3):
            sched[0].setdefault(12 + pt, []).insert(
                0, lambda pt=pt: emit_q(1, pt, tag="kq")
            )
            sched[1][1 + pt] = [lambda pt=pt: emit_q(2, pt)]
            sched[2][1 + pt] = [lambda pt=pt: emit_q(3, pt)]
        for sg in (1, 2, 3):
            for b in range(8):
                sched[sg][4 + b] = [lambda sg=sg, b=b: emit_op(sg - 1, b)]

        pre_ut = {}
        for sg in range(NSTRIPE):
            s0 = sg * SW
            heads_sb[sg] = headsp.tile(
                [P, SW], BF16, tag="heads", bufs=2, name=f"heads{sg}"
            )
            av0 = psum.tile([65, SW], F32, tag="av", bufs=2, name=f"av0_{sg}")
            av1 = psum.tile([65, SW], F32, tag="av", bufs=2, name=f"av1_{sg}")
            ut_tiles = {}
            if sg in pre_ut:
                ut_tiles[0], ut_tiles[1] = pre_ut.pop(sg)

            def emit_av(i):
                ut = ut_tiles.pop(i)
                nc.tensor.matmul(
                    av0[:],
                    vaug_sb[:, i, 0:65],
                    ut[:, 0:SW],
                    start=(i == 0),
                    stop=(i == 15),
                    skip_group_check=True,
                )
                nc.tensor.matmul(
                    av1[:],
                    vaug_sb[:, i, 65:130],
                    ut[:, SW : 2 * SW],
                    start=(i == 0),
                    stop=(i == 15),
                    skip_group_check=True,
                )

            def emit_st_exp(sg_, i_):
                t0 = i_ * P
                s0_ = sg_ * SW
                st = psum.tile(
                    [P, 2 * SW], F32, tag="st", bufs=2, name=f"st{sg_}_{i_}"
                )
                nc.tensor.matmul(
                    st[:, 0:SW],
                    kt_sb[0:64, t0 : t0 + P],
                    qt_sb[0:64, s0_ : s0_ + SW],
                    start=True,
                    stop=True,
                )
                nc.tensor.matmul(
                    st[:, SW : 2 * SW],
                    kt_sb[64:128, t0 : t0 + P],
                    qt_sb[64:128, s0_ : s0_ + SW],
                    start=True,
                    stop=True,
                )
                ut = utp.tile([P, 2 * SW], BF16, tag="ut", bufs=4)
                nc.scalar.activation(ut[:], st[:], EXP, scale=EXP_SCALE)
                return ut

            for i in range(16):
                if i not in ut_tiles:
                    ut_tiles[i] = emit_st_exp(sg, i)
                ut = ut_tiles[i]
                units = sched[sg].get(i, ())
                for u in units:
                    u()
                if sg > 0 and not units:
                    emit_warm(1, moving=ut[:, 0:256], nw=256)
                if i == 15 and sg + 1 < NSTRIPE:
                    pre_ut[sg + 1] = (
                        emit_st_exp(sg + 1, 0),
                        emit_st_exp(sg + 1, 1),
                    )
                if i > 0:
                    emit_av(i - 1)
            emit_av(15)

            # ---- normalize: single copies release the av banks; both
            # heads' denominator rows go through one DMA-reshaped [128, 8]
            # reciprocal, then broadcast back and multiply into heads_sb.
            un0 = normp.tile([65, SW], F32, tag="un", bufs=2, name=f"un0_{sg}")
            nc.vector.tensor_copy(un0[:], av0[:])
            un1 = normp.tile([65, SW], F32, tag="un", bufs=2, name=f"un1_{sg}")
            nc.vector.tensor_copy(un1[:], av1[:])
            e0 = nc.scalar if sg == NSTRIPE - 1 else nc.sync
            e1 = nc.scalar if sg == NSTRIPE - 1 else nc.gpsimd
            rd = dramsm.tile([2, SW], F32, tag="rd", bufs=2)
            e0.dma_start(rd[0:1, :], un0[64:65, :])
            e1.dma_start(rd[1:2, :], un1[64:65, :])
            rb0 = normp.tile([64, SW], F32, tag="rb", bufs=2, name=f"rb0_{sg}")
            e0.dma_start(rb0[:], rd[0:1, :].to_broadcast((64, SW)))
            rb1 = normp.tile([64, SW], F32, tag="rb", bufs=2, name=f"rb1_{sg}")
            e1.dma_start(rb1[:], rd[1:2, :].to_broadcast((64, SW)))
            nc.vector.reciprocal_approx_fast(rb0[:], rb0[:])
            nc.vector.tensor_tensor(
                heads_sb[sg][0:64, :], un0[0:64, :], rb0[:], MULT
            )
            nc.vector.reciprocal_approx_fast(rb1[:], rb1[:])
            nc.vector.tensor_tensor(
                heads_sb[sg][64:128, :], un1[0:64, :], rb1[:], MULT
            )

        # ---- epilogue: dummy matmuls keep the PE clock warm through the
        # last normalize, then the final outproj rotates over freed banks.
        emit_warm(10)
        rot = [("st", 2), ("st", 2), ("vop", 1), ("kq", 1)]
        for blk in range(8):
            tag, bufs = rot[blk % 4]
            emit_op(3, blk, tag=tag, bufs=bufs, scalar_bias=(blk % 2 == 1),
                    engs=(nc.sync, nc.gpsimd, nc.scalar))

    nc.finalize()
    return nc


_NC_CACHE = None


def _get_nc():
    global _NC_CACHE
    if _NC_CACHE is None:
        _NC_CACHE = _build_nc()
    return _NC_CACHE


def _make_in_maps(embeddings, Wq, bq, Wk, bk, Wv, bv, Wo, bo):
    import ml_dtypes

    bf16 = np.dtype(ml_dtypes.bfloat16)
    fp8 = np.dtype(ml_dtypes.float8_e3m4)
    # [1024, 2048] -> [128 pi, 4 chunk, 8 po, 512]: partition pi of d-block
    # po, column chunk c.  (d = po*128 + pi, s = c*512 + sc)
    etT = embeddings.T.astype(np.float32)  # [1024, 2048]
    et4 = np.ascontiguousarray(
        etT.reshape(8, P, NSTRIPE, SW).transpose(1, 2, 0, 3).reshape(P, -1)
    ).astype(fp8)
    in_maps = []
    for c in range(NCORES):
        hs = [2 * c, 2 * c + 1]
        # column order K | V | Q, pre-scaled x64 for fp8 range
        wcat = np.concatenate(
            [Wk[hs[0]], Wk[hs[1]], Wv[hs[0]], Wv[hs[1]], Wq[hs[0]], Wq[hs[1]]],
            axis=1,
        ) * WSCALE  # [1024, 384]
        wqkv_r = np.ascontiguousarray(
            wcat.reshape(8, P, 6 * DK).transpose(1, 0, 2).reshape(P, -1)
        ).astype(fp8)
        bqk = np.stack(
            [np.concatenate([bq[hs[0]], bq[hs[1]]]),
             np.concatenate([bk[hs[0]], bk[hs[1]]])],
            axis=1,
        ).astype(np.float32) * WSCALE  # [128, 2]
        bvc = np.ascontiguousarray(
            np.broadcast_to(
                np.concatenate([bv[hs[0]], bv[hs[1]]])[None, :] * WSCALE,
                (P, 2 * DK),
            ),
            dtype=np.float32,
        )
        bo_eff = bo if c == 0 else np.zeros_like(bo)
        in_maps.append(
            {
                "et": et4,
                "wqkv": wqkv_r,
                "bqk": np.ascontiguousarray(bqk),
                "bv": bvc,
                "bo": np.ascontiguousarray(bo_eff.reshape(8, P).T, dtype=np.float32),
                "wo": np.ascontiguousarray(
                    (Wo[c * P : (c + 1) * P] / WSCALE).astype(bf16)
                ),
            }
        )
    return in_maps


def kernel(embeddings, Wq, bq, Wk, bk, Wv, bv, Wo, bo, **run_kwargs):
    """Full-input / full-output MHA. Shards across 8 NeuronCores internally."""
    nc = _get_nc()
    in_maps = _make_in_maps(
        np.asarray(embeddings, np.float32),
        np.asarray(Wq, np.float32),
        np.asarray(bq, np.float32),
        np.asarray(Wk, np.float32),
        np.asarray(bk, np.float32),
        np.asarray(Wv, np.float32),
        np.asarray(bv, np.float32),
        np.asarray(Wo, np.float32),
        np.asarray(bo, np.float32),
    )
    res = run_bass_kernel_spmd(nc, in_maps, list(range(NCORES)), **run_kwargs)
    # Unshard the row-parallel output projection: sum the per-core bf16
    # partials in fp32, then undo the on-chip out^T layout.
    acc = res.results[0]["out"].astype(np.float32)
    for r_ in res.results[1:]:
        acc += r_["out"].astype(np.float32)
    return np.ascontiguousarray(acc.T)


if __name__ == "__main__":
    rng = np.random.default_rng(0)
    emb = rng.standard_normal((S, D), dtype=np.float32)
    mk = lambda *sh: (rng.standard_normal(sh, dtype=np.float32) * 0.02)
    o = kernel(
        embeddings=emb,
        Wq=mk(H, D, DK), bq=mk(H, DK),
        Wk=mk(H, D, DK), bk=mk(H, DK),
        Wv=mk(H, D, DK), bv=mk(H, DK),
        Wo=mk(H * DK, D), bo=mk(D),
    )
    print(o.shape, o.dtype)
